# revision 11
# baseline (speedup 1.0000x reference)
"""Angular tensor-product basis expansion on 8 Trainium2 NeuronCores.

Input dr [200000, 3] f32 -> output [200000, 1093] f32 where the columns are
the levels of the recursive tensor-product basis: level l has 3^l entries,
entry (j*3+k) of level l = level_{l-1}[j] * dr[k].

The tensor-product basis is symmetric: the level-l entry with base-3 digits
(d1..dl) equals x^a y^b z^c where a,b,c count the digits equal to 0,1,2.
Level l therefore has only C(l+2,2) distinct values; across levels 0..6 the
1093 columns take just 84 distinct monomial values per row, and 4 of those
(1, x, y, z) are the input itself. The device computes exactly the 80
level-2..6 monomials per row (bf16) and the host expands them to the full
1093 fp32 columns with a precomputed index gather during the unshard step --
cutting HBM store traffic per core from 109.7 MB (fp32 full) to 4.0 MB, a
27x reduction on the memory-bound store stream.

Monomial ordering (so each level needs only 3 strided DVE ops):
  L_1 = [x, y, z];  L_l = [x * L_{l-1} (all)] ++ [y * (last l of L_{l-1})]
                          ++ [z * (last 1 of L_{l-1})]
By induction the a=0 monomials are exactly the trailing l+1 entries of L_l,
so the y-source (a=0 entries of L_{l-1}) is a contiguous tail slice.
Level 2 reads x,y,z straight from the input tile.

Measured DVE cost: op duration ~ n_runs * (run_len * ~1.05ns + ~2.6ns) with
a ~170ns issue floor, where a "run" is the op's innermost contiguous span.
So each chunk's scratch tile is MONOMIAL-MAJOR ([partition, monomial, row]):
every op's inner run is the row dimension (28..70 elems), not the 1..21
monomials a row-major layout would give -- this matters most for the tiny
z-power and y-tail ops, which in row-major cost ~2.6-5ns per element.
Chunks are processed in interleaved pairs so every RAW wait (ops are not
interlocked; each op's completion tick is what dependents wait on) lands
4+ ops after its producer and is pre-satisfied. Store DMAs are contiguous
per-partition dumps of the tile (the host untangles the per-chunk transpose
during the gather), alternating between the sync and scalar DMA queues,
which together sustain >400 GB/s. A second compute engine does not help:
vector and gpsimd contend for the same bandwidth and gpsimd's per-op cost
is ~3x.

Data-parallel row sharding across 8 cores (25000 rows each, padded to
25088 = 128 partitions * 196 rows). Partition p owns the contiguous row
chunk [p*196, (p+1)*196).

Raw Bass (no Tile) so DMA instructions carry at most one semaphore wait --
walrus rejects HWDGE direct DMAs with more than one sync-wait command.
"""

import numpy as np

L_MAX = 6
N_CORES = 8
G = 196  # rows owned by one partition
ROWS_PER_CORE = 128 * G  # 25088
S = [1, 3, 6, 10, 15, 21, 28]  # unique monomials per level
OFF = [0, 0, 0, 6, 16, 31, 52]  # device column offset of level l (l>=2)
U = 80  # stored monomials (levels 2..6)
SIZES = (70, 70, 28, 28)  # rows per chunk; consecutive pairs interleave
POPS = 30  # vector ops per pair: 5 levels * (z_a z_b B_a B_b A_a A_b)


def _index_map():
    """Map each of the 1093 reference columns to unique-monomial index 0..83
    (0..3 = [1, x, y, z] host-side; 4+i = device column i)."""
    mono = [[(0, 0, 0)]]
    for l in range(1, L_MAX + 1):
        prev = mono[-1]
        cur = [(a + 1, b, c) for (a, b, c) in prev]
        cur += [(a, b + 1, c) for (a, b, c) in prev[-l:]]
        a, b, c = prev[-1]
        cur += [(a, b, c + 1)]
        mono.append(cur)
    lookup = {t: i for i, t in enumerate(t for lst in mono for t in lst)}
    idx = []
    for l in range(L_MAX + 1):
        for j in range(3**l):
            a = b = c = 0
            for _ in range(l):
                d = j % 3
                j //= 3
                a += d == 0
                b += d == 1
                c += d == 2
            idx.append(lookup[(a, b, c)])
    return np.asarray(idx, dtype=np.intp)


IDX = _index_map()  # [1093] into [1, x, y, z, device cols 0..79]


def _build_nc(sizes=SIZES):
    import concourse.bass as bass
    import concourse.mybir as mybir

    bf16 = mybir.dt.bfloat16
    g = sum(sizes)
    assert g == G
    rows = 128 * g
    starts = np.concatenate([[0], np.cumsum(sizes)[:-1]])
    n_ch = len(sizes)
    assert n_ch % 2 == 0

    nc = bass.Bass()
    dr4 = nc.declare_dram_parameter("dr4", [rows, 4], bf16, isOutput=False)
    # per chunk k the dump is [p, monomial c, row t]: element (p, k, c, t)
    # lands at out[p, starts[k]*U + c*sizes[k] + t]; host untangles
    out = nc.declare_dram_parameter("out", [128, g * U], bf16, isOutput=True)

    dr4_v = dr4[:, :].rearrange("(p g) c -> p (g c)", p=128)  # [128, g*4]

    from contextlib import ExitStack

    with ExitStack() as stack:
        drt = stack.enter_context(nc.sbuf_tensor("drt", [128, g * 4], bf16))
        uq = stack.enter_context(nc.sbuf_tensor("uq", [128, g * U], bf16))
        sem_in = stack.enter_context(nc.semaphore("sem_in"))
        sem_in2 = stack.enter_context(nc.semaphore("sem_in2"))
        sem_out = stack.enter_context(nc.semaphore("sem_out"))
        sem_out2 = stack.enter_context(nc.semaphore("sem_out2"))
        sem_v = stack.enter_context(nc.semaphore("sem_v"))
        block = stack.enter_context(nc.Block(no_gpsimd_drain=True))

        def cview(k):
            # chunk k scratch as [p, monomial, row]
            st, sz = starts[k], sizes[k]
            return uq[:, st * U : (st + sz) * U].rearrange(
                "p (c t) -> p c t", c=U
            )

        def dcomp(k, c0, c1):
            # input components [c0, c1) for chunk k as [p, comp, row]
            st, sz = starts[k], sizes[k]
            return drt[:, st * 4 : (st + sz) * 4].rearrange(
                "p (t c) -> p c t", c=4
            )[:, c0:c1, :]

        # pair op order per level: z_a z_b B_a B_b A_a A_b (6 per level);
        # chunk a completes at its A6 (pair index 29), chunk b at 30
        def cthr(k):
            return POPS * (k // 2) + 29 + (k % 2)

        def store(q, k, sem):
            st, sz = starts[k], sizes[k]
            q.wait_ge(sem_v, cthr(k))
            q.dma_start(
                out=out[:, st * U : (st + sz) * U],
                in_=uq[:, st * U : (st + sz) * U],
            ).then_inc(sem, 16)

        @block.sync
        def _(sync):
            c0 = (sizes[0] + sizes[1]) * 4  # first-pair input columns
            sync.dma_start(out=drt[:, :c0], in_=dr4_v[:, :c0]).then_inc(
                sem_in, 16
            )
            sync.dma_start(out=drt[:, c0:], in_=dr4_v[:, c0:]).then_inc(
                sem_in2, 16
            )
            for k in range(0, n_ch, 2):  # even chunks on the sync queue
                store(sync, k, sem_out)
            sync.wait_ge(sem_out, 16 * (n_ch // 2))

        @block.scalar
        def _(scalar):
            for k in range(1, n_ch, 2):  # odd chunks on the scalar queue
                store(scalar, k, sem_out2)
            scalar.wait_ge(sem_out2, 16 * (n_ch - n_ch // 2))

        @block.vector
        def _(vector):
            vector.wait_ge(sem_in, 16)
            for pair in range(n_ch // 2):
                if pair == 1:
                    vector.wait_ge(sem_in2, 16)
                ks = (2 * pair, 2 * pair + 1)
                vs = [cview(k) for k in ks]
                base = POPS * pair

                for l in range(2, L_MAX + 1):
                    lb = base + 6 * (l - 3)  # previous level's block base
                    o, ps = OFF[l], S[l - 1]
                    zo = o + ps + l  # z^l slot (last of level l)
                    po = OFF[l - 1]
                    pzo = po + ps - 1
                    pto = po + ps - l
                    # z^l = z * z^(l-1)
                    for j, k in enumerate(ks):
                        vv, sz = vs[j], sizes[k]
                        if l == 2:
                            zin = dcomp(k, 3, 4)
                        else:
                            vector.wait_ge(sem_v, lb + 1 + j)
                            zin = vv[:, pzo : pzo + 1, :]
                        vector.tensor_mul(
                            out=vv[:, zo : zo + 1, :],
                            in0=zin,
                            in1=dcomp(k, 3, 4),
                        ).then_inc(sem_v, 1)
                    # y * (a=0 tail of L_{l-1}: its last l entries)
                    for j, k in enumerate(ks):
                        vv, sz = vs[j], sizes[k]
                        if l == 2:
                            ysrc = dcomp(k, 2, 4)
                        else:
                            vector.wait_ge(sem_v, lb + 3 + j)
                            ysrc = vv[:, pto : pto + l, :]
                        vector.tensor_mul(
                            out=vv[:, o + ps : o + ps + l, :],
                            in0=ysrc,
                            in1=dcomp(k, 2, 3).broadcast_to([128, l, sz]),
                        ).then_inc(sem_v, 1)
                    # x * (all of L_{l-1})
                    for j, k in enumerate(ks):
                        vv, sz = vs[j], sizes[k]
                        if l == 2:
                            prev = dcomp(k, 1, 4)
                        else:
                            vector.wait_ge(sem_v, lb + 5 + j)
                            prev = vv[:, po : po + ps, :]
                        vector.tensor_mul(
                            out=vv[:, o : o + ps, :],
                            in0=prev,
                            in1=dcomp(k, 1, 2).broadcast_to([128, ps, sz]),
                        ).then_inc(sem_v, 1)

    return nc


def kernel(dr, _trace=False, _trace_cores=None):
    import ml_dtypes
    from concourse.bass_utils import run_bass_kernel_spmd

    dr = np.asarray(dr, dtype=np.float32)
    n = dr.shape[0]
    # Overlapping shards: core i processes rows [i*step, i*step + 25088) so
    # the 704 rows of pad-to-25088 waste is spread evenly (88 rows per core)
    # instead of all landing on the last core.
    step = n // N_CORES
    assert step <= ROWS_PER_CORE and (N_CORES - 1) * step + ROWS_PER_CORE >= n
    total = (N_CORES - 1) * step + ROWS_PER_CORE
    drb = dr.astype(ml_dtypes.bfloat16)
    dr4 = np.zeros((total, 4), dtype=ml_dtypes.bfloat16)
    dr4[:, 0] = 1.0
    dr4[:n, 1:] = drb

    in_maps = [
        {"dr4": np.ascontiguousarray(dr4[i * step : i * step + ROWS_PER_CORE])}
        for i in range(N_CORES)
    ]
    nc = _build_nc()
    res = run_bass_kernel_spmd(
        nc,
        in_maps,
        core_ids=list(range(N_CORES)),
        trace=_trace,
        trace_cores=_trace_cores,
    )
    kernel.last_result = res

    # untangle the monomial-major per-chunk dumps into [25088, 80] per core
    starts = np.concatenate([[0], np.cumsum(SIZES)[:-1]])
    per_core = []
    for i in range(N_CORES):
        arr = np.asarray(res.results[i]["out"])  # [128, G*U] bf16
        blocks = []
        for k, sz in enumerate(SIZES):
            b = arr[:, starts[k] * U : (starts[k] + sz) * U]
            blocks.append(b.reshape(128, U, sz).transpose(0, 2, 1))
        per_core.append(
            np.concatenate(blocks, axis=1).reshape(ROWS_PER_CORE, U)
        )
    dev = np.concatenate(
        [per_core[i][:step] for i in range(N_CORES - 1)]
        + [per_core[N_CORES - 1][: ROWS_PER_CORE - 88]],
        axis=0,
    )
    # unshard: assemble the 84 unique monomials (host-known [1,x,y,z] +
    # 80 device columns), upcast, and expand to the 1093 output columns
    uniq = np.empty((n, 84), dtype=np.float32)
    uniq[:, 0] = 1.0
    uniq[:, 1:4] = drb.astype(np.float32)  # match device bf16 rounding
    uniq[:, 4:] = dev[:n].astype(np.float32)
    return uniq[:, IDX]


# revision 13
# speedup vs baseline: 1.0584x; 1.0584x over previous
"""Angular tensor-product basis expansion on 8 Trainium2 NeuronCores.

Input dr [200000, 3] f32 -> output [200000, 1093] f32 where the columns are
the levels of the recursive tensor-product basis: level l has 3^l entries,
entry (j*3+k) of level l = level_{l-1}[j] * dr[k].

The tensor-product basis is symmetric: the level-l entry with base-3 digits
(d1..dl) equals x^a y^b z^c where a,b,c count the digits equal to 0,1,2.
Level l therefore has only C(l+2,2) distinct values; across levels 0..6 the
1093 columns take just 84 distinct monomial values per row, and 4 of those
(1, x, y, z) are the input itself. The device computes exactly the 80
level-2..6 monomials per row (bf16) and the host expands them to the full
1093 fp32 columns with a precomputed index gather during the unshard step --
cutting HBM store traffic per core from 109.7 MB (fp32 full) to 4.0 MB, a
27x reduction on the memory-bound store stream.

Monomial ordering (so each level needs only 3 strided DVE ops):
  L_1 = [x, y, z];  L_l = [x * L_{l-1} (all)] ++ [y * (last l of L_{l-1})]
                          ++ [z * (last 1 of L_{l-1})]
By induction the a=0 monomials are exactly the trailing l+1 entries of L_l,
so the y-source (a=0 entries of L_{l-1}) is a contiguous tail slice.
Level 2 reads x,y,z straight from the input tile.

Measured DVE cost: op duration ~ n_runs * (run_len * ~1.05ns + ~2.6ns) with
a ~170ns issue floor, where a "run" is the op's innermost contiguous span.
So each chunk's scratch tile is MONOMIAL-MAJOR ([partition, monomial, row]):
every op's inner run is the row dimension (28..70 elems), not the 1..21
monomials a row-major layout would give -- this matters most for the tiny
z-power and y-tail ops, which in row-major cost ~2.6-5ns per element.
Chunks are processed in interleaved pairs so every RAW wait (ops are not
interlocked; each op's completion tick is what dependents wait on) lands
4+ ops after its producer and is pre-satisfied. Store DMAs are contiguous
per-partition dumps of the tile (the host untangles the per-chunk transpose
during the gather), alternating between the sync and scalar DMA queues,
which together sustain >400 GB/s. A second compute engine does not help:
vector and gpsimd contend for the same bandwidth and gpsimd's per-op cost
is ~3x.

Data-parallel row sharding across 8 cores (25000 rows each, padded to
25088 = 128 partitions * 196 rows). Partition p owns the contiguous row
chunk [p*196, (p+1)*196).

Raw Bass (no Tile) so DMA instructions carry at most one semaphore wait --
walrus rejects HWDGE direct DMAs with more than one sync-wait command.
"""

import numpy as np

L_MAX = 6
N_CORES = 8
G = 196  # rows owned by one partition
ROWS_PER_CORE = 128 * G  # 25088
S = [1, 3, 6, 10, 15, 21, 28]  # unique monomials per level
OFF = [0, 0, 0, 6, 16, 31, 52]  # device column offset of level l (l>=2)
U = 80  # stored monomials (levels 2..6)
SIZES = (70, 70, 28, 28)  # rows per chunk; consecutive pairs interleave
POPS = 30  # vector ops per pair: 5 levels * (z_a z_b B_a B_b A_a A_b)


def _index_map():
    """Map each of the 1093 reference columns to unique-monomial index 0..83
    (0..3 = [1, x, y, z] host-side; 4+i = device column i)."""
    mono = [[(0, 0, 0)]]
    for l in range(1, L_MAX + 1):
        prev = mono[-1]
        cur = [(a + 1, b, c) for (a, b, c) in prev]
        cur += [(a, b + 1, c) for (a, b, c) in prev[-l:]]
        a, b, c = prev[-1]
        cur += [(a, b, c + 1)]
        mono.append(cur)
    lookup = {t: i for i, t in enumerate(t for lst in mono for t in lst)}
    idx = []
    for l in range(L_MAX + 1):
        for j in range(3**l):
            a = b = c = 0
            for _ in range(l):
                d = j % 3
                j //= 3
                a += d == 0
                b += d == 1
                c += d == 2
            idx.append(lookup[(a, b, c)])
    return np.asarray(idx, dtype=np.intp)


IDX = _index_map()  # [1093] into [1, x, y, z, device cols 0..79]


def _build_nc(sizes=SIZES):
    import concourse.bass as bass
    import concourse.mybir as mybir

    bf16 = mybir.dt.bfloat16
    g = sum(sizes)
    assert g == G
    rows = 128 * g
    starts = np.concatenate([[0], np.cumsum(sizes)[:-1]])
    n_ch = len(sizes)
    assert n_ch % 2 == 0

    nc = bass.Bass()
    dr4 = nc.declare_dram_parameter("dr4", [rows, 4], bf16, isOutput=False)
    # per chunk k the dump is [p, monomial c, row t]: element (p, k, c, t)
    # lands at out[p, starts[k]*U + c*sizes[k] + t]; host untangles
    out = nc.declare_dram_parameter("out", [128, g * U], bf16, isOutput=True)

    dr4_v = dr4[:, :].rearrange("(p g) c -> p (g c)", p=128)  # [128, g*4]

    from contextlib import ExitStack

    with ExitStack() as stack:
        drt = stack.enter_context(nc.sbuf_tensor("drt", [128, g * 4], bf16))
        uq = stack.enter_context(nc.sbuf_tensor("uq", [128, g * U], bf16))
        sem_in = stack.enter_context(nc.semaphore("sem_in"))
        sem_in2 = stack.enter_context(nc.semaphore("sem_in2"))
        sem_out = stack.enter_context(nc.semaphore("sem_out"))
        sem_out2 = stack.enter_context(nc.semaphore("sem_out2"))
        sem_v = stack.enter_context(nc.semaphore("sem_v"))
        block = stack.enter_context(nc.Block(no_gpsimd_drain=True))

        def cview(k):
            # chunk k scratch as [p, monomial, row]
            st, sz = starts[k], sizes[k]
            return uq[:, st * U : (st + sz) * U].rearrange(
                "p (c t) -> p c t", c=U
            )

        def dcomp(k, c0, c1):
            # input components [c0, c1) for chunk k as [p, comp, row]
            st, sz = starts[k], sizes[k]
            return drt[:, st * 4 : (st + sz) * 4].rearrange(
                "p (t c) -> p c t", c=4
            )[:, c0:c1, :]

        # pair op order per level: z_a z_b B_a B_b A_a A_b (6 per level);
        # chunk a completes at its A6 (pair index 29), chunk b at 30
        def cthr(k):
            return POPS * (k // 2) + 29 + (k % 2)

        def store(q, k, sem, half=None):
            st, sz = starts[k], sizes[k]
            lo, hi = st * U, (st + sz) * U
            if half is not None:  # split one chunk's store across queues
                mid = lo + (hi - lo) // 2
                lo, hi = (lo, mid) if half == 0 else (mid, hi)
            q.wait_ge(sem_v, cthr(k))
            q.dma_start(
                out=out[:, lo:hi], in_=uq[:, lo:hi]
            ).then_inc(sem, 16)

        # the final pair's chunks finish last (nothing left to overlap
        # them with), so split each of their stores across both queues
        @block.sync
        def _(sync):
            c0 = (sizes[0] + sizes[1]) * 4  # first-pair input columns
            sync.dma_start(out=drt[:, :c0], in_=dr4_v[:, :c0]).then_inc(
                sem_in, 16
            )
            sync.dma_start(out=drt[:, c0:], in_=dr4_v[:, c0:]).then_inc(
                sem_in2, 16
            )
            for k in range(0, n_ch - 2, 2):
                store(sync, k, sem_out)
            store(sync, n_ch - 2, sem_out, half=0)
            store(sync, n_ch - 1, sem_out, half=0)
            sync.wait_ge(sem_out, 16 * (n_ch // 2 + 1))

        @block.scalar
        def _(scalar):
            for k in range(1, n_ch - 2, 2):
                store(scalar, k, sem_out2)
            store(scalar, n_ch - 2, sem_out2, half=1)
            store(scalar, n_ch - 1, sem_out2, half=1)
            scalar.wait_ge(sem_out2, 16 * (n_ch // 2 + 1))

        @block.vector
        def _(vector):
            vector.wait_ge(sem_in, 16)
            for pair in range(n_ch // 2):
                if pair == 1:
                    vector.wait_ge(sem_in2, 16)
                ks = (2 * pair, 2 * pair + 1)
                vs = [cview(k) for k in ks]
                base = POPS * pair

                for l in range(2, L_MAX + 1):
                    lb = base + 6 * (l - 3)  # previous level's block base
                    o, ps = OFF[l], S[l - 1]
                    zo = o + ps + l  # z^l slot (last of level l)
                    po = OFF[l - 1]
                    pzo = po + ps - 1
                    pto = po + ps - l
                    # z^l = z * z^(l-1)
                    for j, k in enumerate(ks):
                        vv, sz = vs[j], sizes[k]
                        if l == 2:
                            zin = dcomp(k, 3, 4)
                        else:
                            vector.wait_ge(sem_v, lb + 1 + j)
                            zin = vv[:, pzo : pzo + 1, :]
                        vector.tensor_mul(
                            out=vv[:, zo : zo + 1, :],
                            in0=zin,
                            in1=dcomp(k, 3, 4),
                        ).then_inc(sem_v, 1)
                    # y * (a=0 tail of L_{l-1}: its last l entries)
                    for j, k in enumerate(ks):
                        vv, sz = vs[j], sizes[k]
                        if l == 2:
                            ysrc = dcomp(k, 2, 4)
                        else:
                            vector.wait_ge(sem_v, lb + 3 + j)
                            ysrc = vv[:, pto : pto + l, :]
                        vector.tensor_mul(
                            out=vv[:, o + ps : o + ps + l, :],
                            in0=ysrc,
                            in1=dcomp(k, 2, 3).broadcast_to([128, l, sz]),
                        ).then_inc(sem_v, 1)
                    # x * (all of L_{l-1})
                    for j, k in enumerate(ks):
                        vv, sz = vs[j], sizes[k]
                        if l == 2:
                            prev = dcomp(k, 1, 4)
                        else:
                            vector.wait_ge(sem_v, lb + 5 + j)
                            prev = vv[:, po : po + ps, :]
                        vector.tensor_mul(
                            out=vv[:, o : o + ps, :],
                            in0=prev,
                            in1=dcomp(k, 1, 2).broadcast_to([128, ps, sz]),
                        ).then_inc(sem_v, 1)

    return nc


def kernel(dr, _trace=False, _trace_cores=None):
    import ml_dtypes
    from concourse.bass_utils import run_bass_kernel_spmd

    dr = np.asarray(dr, dtype=np.float32)
    n = dr.shape[0]
    # Overlapping shards: core i processes rows [i*step, i*step + 25088) so
    # the 704 rows of pad-to-25088 waste is spread evenly (88 rows per core)
    # instead of all landing on the last core.
    step = n // N_CORES
    assert step <= ROWS_PER_CORE and (N_CORES - 1) * step + ROWS_PER_CORE >= n
    total = (N_CORES - 1) * step + ROWS_PER_CORE
    drb = dr.astype(ml_dtypes.bfloat16)
    dr4 = np.zeros((total, 4), dtype=ml_dtypes.bfloat16)
    dr4[:, 0] = 1.0
    dr4[:n, 1:] = drb

    in_maps = [
        {"dr4": np.ascontiguousarray(dr4[i * step : i * step + ROWS_PER_CORE])}
        for i in range(N_CORES)
    ]
    nc = _build_nc()
    res = run_bass_kernel_spmd(
        nc,
        in_maps,
        core_ids=list(range(N_CORES)),
        trace=_trace,
        trace_cores=_trace_cores,
    )
    kernel.last_result = res

    # untangle the monomial-major per-chunk dumps into [25088, 80] per core
    starts = np.concatenate([[0], np.cumsum(SIZES)[:-1]])
    per_core = []
    for i in range(N_CORES):
        arr = np.asarray(res.results[i]["out"])  # [128, G*U] bf16
        blocks = []
        for k, sz in enumerate(SIZES):
            b = arr[:, starts[k] * U : (starts[k] + sz) * U]
            blocks.append(b.reshape(128, U, sz).transpose(0, 2, 1))
        per_core.append(
            np.concatenate(blocks, axis=1).reshape(ROWS_PER_CORE, U)
        )
    dev = np.concatenate(
        [per_core[i][:step] for i in range(N_CORES - 1)]
        + [per_core[N_CORES - 1][: ROWS_PER_CORE - 88]],
        axis=0,
    )
    # unshard: assemble the 84 unique monomials (host-known [1,x,y,z] +
    # 80 device columns), upcast, and expand to the 1093 output columns
    uniq = np.empty((n, 84), dtype=np.float32)
    uniq[:, 0] = 1.0
    uniq[:, 1:4] = drb.astype(np.float32)  # match device bf16 rounding
    uniq[:, 4:] = dev[:n].astype(np.float32)
    return uniq[:, IDX]


# revision 14
# speedup vs baseline: 1.1063x; 1.0453x over previous
"""Angular tensor-product basis expansion on 8 Trainium2 NeuronCores.

Input dr [200000, 3] f32 -> output [200000, 1093] f32 where the columns are
the levels of the recursive tensor-product basis: level l has 3^l entries,
entry (j*3+k) of level l = level_{l-1}[j] * dr[k].

The tensor-product basis is symmetric: the level-l entry with base-3 digits
(d1..dl) equals x^a y^b z^c where a,b,c count the digits equal to 0,1,2.
Level l therefore has only C(l+2,2) distinct values; across levels 0..6 the
1093 columns take just 84 distinct monomial values per row, and 4 of those
(1, x, y, z) are the input itself. The device computes exactly the 80
level-2..6 monomials per row (bf16) and the host expands them to the full
1093 fp32 columns with a precomputed index gather during the unshard step --
cutting HBM store traffic per core from 109.7 MB (fp32 full) to 4.0 MB, a
27x reduction on the memory-bound store stream.

Monomial ordering (so each level needs only 3 strided DVE ops):
  L_1 = [x, y, z];  L_l = [x * L_{l-1} (all)] ++ [y * (last l of L_{l-1})]
                          ++ [z * (last 1 of L_{l-1})]
By induction the a=0 monomials are exactly the trailing l+1 entries of L_l,
so the y-source (a=0 entries of L_{l-1}) is a contiguous tail slice.
Level 2 reads x,y,z straight from the input tile.

Measured DVE cost: op duration ~ n_runs * (run_len * ~1.05ns + ~2.6ns) with
a ~170ns issue floor, where a "run" is the op's innermost contiguous span.
So each chunk's scratch tile is MONOMIAL-MAJOR ([partition, monomial, row]):
every op's inner run is the row dimension (28..70 elems), not the 1..21
monomials a row-major layout would give -- this matters most for the tiny
z-power and y-tail ops, which in row-major cost ~2.6-5ns per element.
Chunks are processed in interleaved pairs so every RAW wait (ops are not
interlocked; each op's completion tick is what dependents wait on) lands
4+ ops after its producer and is pre-satisfied. Store DMAs are contiguous
per-partition dumps of the tile (the host untangles the per-chunk transpose
during the gather), alternating between the sync and scalar DMA queues,
which together sustain >400 GB/s. A second compute engine does not help:
vector and gpsimd contend for the same bandwidth and gpsimd's per-op cost
is ~3x.

Data-parallel row sharding across 8 cores (25000 rows each, padded to
25088 = 128 partitions * 196 rows). Partition p owns the contiguous row
chunk [p*196, (p+1)*196).

Raw Bass (no Tile) so DMA instructions carry at most one semaphore wait --
walrus rejects HWDGE direct DMAs with more than one sync-wait command.
"""

import numpy as np

L_MAX = 6
N_CORES = 8
G = 196  # rows owned by one partition
ROWS_PER_CORE = 128 * G  # 25088
S = [1, 3, 6, 10, 15, 21, 28]  # unique monomials per level
OFF = [0, 0, 0, 6, 16, 31, 52]  # device column offset of level l (l>=2)
U = 80  # stored monomials (levels 2..6)
SIZES = (70, 70, 28, 28)  # rows per chunk; consecutive pairs interleave
POPS = 30  # vector ops per pair: 5 levels * (z_a z_b B_a B_b A_a A_b)


def _index_map():
    """Map each of the 1093 reference columns to unique-monomial index 0..83
    (0..3 = [1, x, y, z] host-side; 4+i = device column i)."""
    mono = [[(0, 0, 0)]]
    for l in range(1, L_MAX + 1):
        prev = mono[-1]
        cur = [(a + 1, b, c) for (a, b, c) in prev]
        cur += [(a, b + 1, c) for (a, b, c) in prev[-l:]]
        a, b, c = prev[-1]
        cur += [(a, b, c + 1)]
        mono.append(cur)
    lookup = {t: i for i, t in enumerate(t for lst in mono for t in lst)}
    idx = []
    for l in range(L_MAX + 1):
        for j in range(3**l):
            a = b = c = 0
            for _ in range(l):
                d = j % 3
                j //= 3
                a += d == 0
                b += d == 1
                c += d == 2
            idx.append(lookup[(a, b, c)])
    return np.asarray(idx, dtype=np.intp)


IDX = _index_map()  # [1093] into [1, x, y, z, device cols 0..79]


def _build_nc(sizes=SIZES):
    import concourse.bass as bass
    import concourse.mybir as mybir

    bf16 = mybir.dt.bfloat16
    g = sum(sizes)
    assert g == G
    rows = 128 * g
    starts = np.concatenate([[0], np.cumsum(sizes)[:-1]])
    n_ch = len(sizes)
    assert n_ch % 2 == 0

    nc = bass.Bass()
    dr4 = nc.declare_dram_parameter("dr4", [rows, 4], bf16, isOutput=False)
    # per chunk k the dump is [p, monomial c, row t]: element (p, k, c, t)
    # lands at out[p, starts[k]*U + c*sizes[k] + t]; host untangles
    out = nc.declare_dram_parameter("out", [128, g * U], bf16, isOutput=True)

    dr4_v = dr4[:, :].rearrange("(p g) c -> p (g c)", p=128)  # [128, g*4]

    from contextlib import ExitStack

    with ExitStack() as stack:
        drt = stack.enter_context(nc.sbuf_tensor("drt", [128, g * 4], bf16))
        uq = stack.enter_context(nc.sbuf_tensor("uq", [128, g * U], bf16))
        sem_in = stack.enter_context(nc.semaphore("sem_in"))
        sem_in2 = stack.enter_context(nc.semaphore("sem_in2"))
        sem_out = stack.enter_context(nc.semaphore("sem_out"))
        sem_out2 = stack.enter_context(nc.semaphore("sem_out2"))
        sem_v = stack.enter_context(nc.semaphore("sem_v"))
        block = stack.enter_context(nc.Block(no_gpsimd_drain=True))

        def cview(k):
            # chunk k scratch as [p, monomial, row]
            st, sz = starts[k], sizes[k]
            return uq[:, st * U : (st + sz) * U].rearrange(
                "p (c t) -> p c t", c=U
            )

        def dcomp(k, c0, c1):
            # input components [c0, c1) for chunk k as [p, comp, row]
            st, sz = starts[k], sizes[k]
            return drt[:, st * 4 : (st + sz) * 4].rearrange(
                "p (t c) -> p c t", c=4
            )[:, c0:c1, :]

        # pair op order per level: z_a z_b B_a B_b A_a A_b (6 per level);
        # chunk a completes at its A6 (pair index 29), chunk b at 30
        def cthr(k):
            return POPS * (k // 2) + 29 + (k % 2)

        def store(q, k, sem, part=None):
            # part=None: whole chunk, waits full completion. part="lo":
            # levels 2..5 (monomial rows 0..OFF[6]), complete 6 pair-ops
            # before the chunk's level-6 ops; part="hi": level 6 only.
            st, sz = starts[k], sizes[k]
            lo, hi = st * U, (st + sz) * U
            mid = lo + OFF[6] * sz
            if part == "lo":
                hi = mid
                # A5 of this chunk: all its level<=5 ops are complete
                q.wait_ge(sem_v, POPS * (k // 2) + 23 + (k % 2))
            else:
                if part == "hi":
                    lo = mid
                q.wait_ge(sem_v, cthr(k))
            q.dma_start(
                out=out[:, lo:hi], in_=uq[:, lo:hi]
            ).then_inc(sem, 16)

        # the final pair's chunks finish last (nothing left to overlap
        # them with), so stream their level-2..5 bytes early and only the
        # small level-6 block after full completion, one chunk per queue
        @block.sync
        def _(sync):
            c0 = (sizes[0] + sizes[1]) * 4  # first-pair input columns
            sync.dma_start(out=drt[:, :c0], in_=dr4_v[:, :c0]).then_inc(
                sem_in, 16
            )
            sync.dma_start(out=drt[:, c0:], in_=dr4_v[:, c0:]).then_inc(
                sem_in2, 16
            )
            for k in range(0, n_ch - 2, 2):
                store(sync, k, sem_out)
            store(sync, n_ch - 2, sem_out, part="lo")
            store(sync, n_ch - 2, sem_out, part="hi")
            sync.wait_ge(sem_out, 16 * (n_ch // 2 + 1))

        @block.scalar
        def _(scalar):
            for k in range(1, n_ch - 2, 2):
                store(scalar, k, sem_out2)
            store(scalar, n_ch - 1, sem_out2, part="lo")
            store(scalar, n_ch - 1, sem_out2, part="hi")
            scalar.wait_ge(sem_out2, 16 * (n_ch // 2 + 1))

        @block.vector
        def _(vector):
            vector.wait_ge(sem_in, 16)
            for pair in range(n_ch // 2):
                if pair == 1:
                    vector.wait_ge(sem_in2, 16)
                ks = (2 * pair, 2 * pair + 1)
                vs = [cview(k) for k in ks]
                base = POPS * pair

                for l in range(2, L_MAX + 1):
                    lb = base + 6 * (l - 3)  # previous level's block base
                    o, ps = OFF[l], S[l - 1]
                    zo = o + ps + l  # z^l slot (last of level l)
                    po = OFF[l - 1]
                    pzo = po + ps - 1
                    pto = po + ps - l
                    # z^l = z * z^(l-1)
                    for j, k in enumerate(ks):
                        vv, sz = vs[j], sizes[k]
                        if l == 2:
                            zin = dcomp(k, 3, 4)
                        else:
                            vector.wait_ge(sem_v, lb + 1 + j)
                            zin = vv[:, pzo : pzo + 1, :]
                        vector.tensor_mul(
                            out=vv[:, zo : zo + 1, :],
                            in0=zin,
                            in1=dcomp(k, 3, 4),
                        ).then_inc(sem_v, 1)
                    # y * (a=0 tail of L_{l-1}: its last l entries)
                    for j, k in enumerate(ks):
                        vv, sz = vs[j], sizes[k]
                        if l == 2:
                            ysrc = dcomp(k, 2, 4)
                        else:
                            vector.wait_ge(sem_v, lb + 3 + j)
                            ysrc = vv[:, pto : pto + l, :]
                        vector.tensor_mul(
                            out=vv[:, o + ps : o + ps + l, :],
                            in0=ysrc,
                            in1=dcomp(k, 2, 3).broadcast_to([128, l, sz]),
                        ).then_inc(sem_v, 1)
                    # x * (all of L_{l-1})
                    for j, k in enumerate(ks):
                        vv, sz = vs[j], sizes[k]
                        if l == 2:
                            prev = dcomp(k, 1, 4)
                        else:
                            vector.wait_ge(sem_v, lb + 5 + j)
                            prev = vv[:, po : po + ps, :]
                        vector.tensor_mul(
                            out=vv[:, o : o + ps, :],
                            in0=prev,
                            in1=dcomp(k, 1, 2).broadcast_to([128, ps, sz]),
                        ).then_inc(sem_v, 1)

    return nc


def kernel(dr, _trace=False, _trace_cores=None):
    import ml_dtypes
    from concourse.bass_utils import run_bass_kernel_spmd

    dr = np.asarray(dr, dtype=np.float32)
    n = dr.shape[0]
    # Overlapping shards: core i processes rows [i*step, i*step + 25088) so
    # the 704 rows of pad-to-25088 waste is spread evenly (88 rows per core)
    # instead of all landing on the last core.
    step = n // N_CORES
    assert step <= ROWS_PER_CORE and (N_CORES - 1) * step + ROWS_PER_CORE >= n
    total = (N_CORES - 1) * step + ROWS_PER_CORE
    drb = dr.astype(ml_dtypes.bfloat16)
    dr4 = np.zeros((total, 4), dtype=ml_dtypes.bfloat16)
    dr4[:, 0] = 1.0
    dr4[:n, 1:] = drb

    in_maps = [
        {"dr4": np.ascontiguousarray(dr4[i * step : i * step + ROWS_PER_CORE])}
        for i in range(N_CORES)
    ]
    nc = _build_nc()
    res = run_bass_kernel_spmd(
        nc,
        in_maps,
        core_ids=list(range(N_CORES)),
        trace=_trace,
        trace_cores=_trace_cores,
    )
    kernel.last_result = res

    # untangle the monomial-major per-chunk dumps into [25088, 80] per core
    starts = np.concatenate([[0], np.cumsum(SIZES)[:-1]])
    per_core = []
    for i in range(N_CORES):
        arr = np.asarray(res.results[i]["out"])  # [128, G*U] bf16
        blocks = []
        for k, sz in enumerate(SIZES):
            b = arr[:, starts[k] * U : (starts[k] + sz) * U]
            blocks.append(b.reshape(128, U, sz).transpose(0, 2, 1))
        per_core.append(
            np.concatenate(blocks, axis=1).reshape(ROWS_PER_CORE, U)
        )
    dev = np.concatenate(
        [per_core[i][:step] for i in range(N_CORES - 1)]
        + [per_core[N_CORES - 1][: ROWS_PER_CORE - 88]],
        axis=0,
    )
    # unshard: assemble the 84 unique monomials (host-known [1,x,y,z] +
    # 80 device columns), upcast, and expand to the 1093 output columns
    uniq = np.empty((n, 84), dtype=np.float32)
    uniq[:, 0] = 1.0
    uniq[:, 1:4] = drb.astype(np.float32)  # match device bf16 rounding
    uniq[:, 4:] = dev[:n].astype(np.float32)
    return uniq[:, IDX]


# revision 16
# speedup vs baseline: 1.1595x; 1.0480x over previous
"""Angular tensor-product basis expansion on 8 Trainium2 NeuronCores.

Input dr [200000, 3] f32 -> output [200000, 1093] f32 where the columns are
the levels of the recursive tensor-product basis: level l has 3^l entries,
entry (j*3+k) of level l = level_{l-1}[j] * dr[k].

The tensor-product basis is symmetric: the level-l entry with base-3 digits
(d1..dl) equals x^a y^b z^c where a,b,c count the digits equal to 0,1,2.
Level l therefore has only C(l+2,2) distinct values; across levels 0..6 the
1093 columns take just 84 distinct monomial values per row, and 4 of those
(1, x, y, z) are the input itself. The device computes exactly the 80
level-2..6 monomials per row (bf16) and the host expands them to the full
1093 fp32 columns with a precomputed index gather during the unshard step --
cutting HBM store traffic per core from 109.7 MB (fp32 full) to 4.0 MB, a
27x reduction on the memory-bound store stream.

Monomial ordering (so each level needs only 3 strided DVE ops):
  L_1 = [x, y, z];  L_l = [x * L_{l-1} (all)] ++ [y * (last l of L_{l-1})]
                          ++ [z * (last 1 of L_{l-1})]
By induction the a=0 monomials are exactly the trailing l+1 entries of L_l,
so the y-source (a=0 entries of L_{l-1}) is a contiguous tail slice.
Level 2 reads x,y,z straight from the input tile.

Measured DVE cost: op duration ~ n_runs * (run_len * ~1.05ns + ~2.6ns) with
a ~170ns issue floor, where a "run" is the op's innermost contiguous span.
So each chunk's scratch tile is MONOMIAL-MAJOR ([partition, monomial, row]):
every op's inner run is the row dimension (28..70 elems), not the 1..21
monomials a row-major layout would give -- this matters most for the tiny
z-power and y-tail ops, which in row-major cost ~2.6-5ns per element.
Chunks are processed in interleaved pairs so every RAW wait (ops are not
interlocked; each op's completion tick is what dependents wait on) lands
4+ ops after its producer and is pre-satisfied. Store DMAs are contiguous
per-partition dumps of the tile (the host untangles the per-chunk transpose
during the gather), alternating between the sync and scalar DMA queues,
which together sustain >400 GB/s. A second compute engine does not help:
vector and gpsimd contend for the same bandwidth and gpsimd's per-op cost
is ~3x.

Data-parallel row sharding across 8 cores (25000 rows each, padded to
25088 = 128 partitions * 196 rows). Partition p owns the contiguous row
chunk [p*196, (p+1)*196).

Raw Bass (no Tile) so DMA instructions carry at most one semaphore wait --
walrus rejects HWDGE direct DMAs with more than one sync-wait command.
"""

import numpy as np

L_MAX = 6
N_CORES = 8
G = 196  # rows owned by one partition
ROWS_PER_CORE = 128 * G  # 25088
S = [1, 3, 6, 10, 15, 21, 28]  # unique monomials per level
OFF = [0, 0, 0, 6, 16, 31, 52]  # device column offset of level l (l>=2)
U = 80  # stored monomials (levels 2..6)
SIZES = (98, 98)  # rows per chunk; consecutive pairs interleave
POPS = 30  # vector ops per pair: 5 levels * (z_a z_b B_a B_b A_a A_b)


def _index_map():
    """Map each of the 1093 reference columns to unique-monomial index 0..83
    (0..3 = [1, x, y, z] host-side; 4+i = device column i)."""
    mono = [[(0, 0, 0)]]
    for l in range(1, L_MAX + 1):
        prev = mono[-1]
        cur = [(a + 1, b, c) for (a, b, c) in prev]
        cur += [(a, b + 1, c) for (a, b, c) in prev[-l:]]
        a, b, c = prev[-1]
        cur += [(a, b, c + 1)]
        mono.append(cur)
    lookup = {t: i for i, t in enumerate(t for lst in mono for t in lst)}
    idx = []
    for l in range(L_MAX + 1):
        for j in range(3**l):
            a = b = c = 0
            for _ in range(l):
                d = j % 3
                j //= 3
                a += d == 0
                b += d == 1
                c += d == 2
            idx.append(lookup[(a, b, c)])
    return np.asarray(idx, dtype=np.intp)


IDX = _index_map()  # [1093] into [1, x, y, z, device cols 0..79]


def _build_nc(sizes=SIZES):
    import concourse.bass as bass
    import concourse.mybir as mybir

    bf16 = mybir.dt.bfloat16
    g = sum(sizes)
    assert g == G
    rows = 128 * g
    starts = np.concatenate([[0], np.cumsum(sizes)[:-1]])
    n_ch = len(sizes)
    assert n_ch % 2 == 0

    nc = bass.Bass()
    dr4 = nc.declare_dram_parameter("dr4", [rows, 4], bf16, isOutput=False)
    # per chunk k the dump is [p, monomial c, row t]: element (p, k, c, t)
    # lands at out[p, starts[k]*U + c*sizes[k] + t]; host untangles
    out = nc.declare_dram_parameter("out", [128, g * U], bf16, isOutput=True)

    dr4_v = dr4[:, :].rearrange("(p g) c -> p (g c)", p=128)  # [128, g*4]

    from contextlib import ExitStack

    with ExitStack() as stack:
        drt = stack.enter_context(nc.sbuf_tensor("drt", [128, g * 4], bf16))
        uq = stack.enter_context(nc.sbuf_tensor("uq", [128, g * U], bf16))
        sem_in = stack.enter_context(nc.semaphore("sem_in"))
        sem_in2 = stack.enter_context(nc.semaphore("sem_in2"))
        sem_out = stack.enter_context(nc.semaphore("sem_out"))
        sem_out2 = stack.enter_context(nc.semaphore("sem_out2"))
        sem_v = stack.enter_context(nc.semaphore("sem_v"))
        block = stack.enter_context(nc.Block(no_gpsimd_drain=True))

        def cview(k):
            # chunk k scratch as [p, monomial, row]
            st, sz = starts[k], sizes[k]
            return uq[:, st * U : (st + sz) * U].rearrange(
                "p (c t) -> p c t", c=U
            )

        def dcomp(k, c0, c1):
            # input components [c0, c1) for chunk k as [p, comp, row]
            st, sz = starts[k], sizes[k]
            return drt[:, st * 4 : (st + sz) * 4].rearrange(
                "p (t c) -> p c t", c=4
            )[:, c0:c1, :]

        # pair op order per level: z_a z_b B_a B_b A_a A_b (6 per level);
        # chunk a completes at its A6 (pair index 29), chunk b at 30
        def cthr(k):
            return POPS * (k // 2) + 29 + (k % 2)

        def store(q, k, sem, band=None):
            # band=None: whole chunk, waits full completion. Banded stores
            # stream a finished level range early: "lo4" = levels 2..4
            # (ready at the chunk's A4 op), "lo5" = level 5 (at A5),
            # "hi" = level 6 (at full completion).
            st, sz = starts[k], sizes[k]
            base, pj = POPS * (k // 2), k % 2
            lo, hi = st * U, (st + sz) * U
            if band == "lo4":
                hi = lo + OFF[5] * sz
                q.wait_ge(sem_v, base + 17 + pj)
            elif band == "lo5":
                lo, hi = lo + OFF[5] * sz, lo + OFF[6] * sz
                q.wait_ge(sem_v, base + 23 + pj)
            else:
                if band == "hi":
                    lo = lo + OFF[6] * sz
                q.wait_ge(sem_v, cthr(k))
            q.dma_start(
                out=out[:, lo:hi], in_=uq[:, lo:hi]
            ).then_inc(sem, 16)

        # the final pair's chunks finish last (nothing left to overlap
        # them with), so stream their finished level bands early and only
        # the small level-6 block after completion, one chunk per queue
        sync_jobs = [(k, None) for k in range(0, n_ch - 2, 2)]
        sync_jobs += [(n_ch - 2, "lo4"), (n_ch - 2, "lo5"), (n_ch - 2, "hi")]
        scalar_jobs = [(k, None) for k in range(1, n_ch - 2, 2)]
        scalar_jobs += [(n_ch - 1, "lo4"), (n_ch - 1, "lo5"), (n_ch - 1, "hi")]

        @block.sync
        def _(sync):
            if n_ch > 2:
                c0 = (sizes[0] + sizes[1]) * 4  # first-pair input columns
                sync.dma_start(out=drt[:, :c0], in_=dr4_v[:, :c0]).then_inc(
                    sem_in, 16
                )
                sync.dma_start(out=drt[:, c0:], in_=dr4_v[:, c0:]).then_inc(
                    sem_in2, 16
                )
            else:
                sync.dma_start(out=drt[:, :], in_=dr4_v).then_inc(sem_in, 16)
            for k, band in sync_jobs:
                store(sync, k, sem_out, band)
            sync.wait_ge(sem_out, 16 * len(sync_jobs))

        @block.scalar
        def _(scalar):
            for k, band in scalar_jobs:
                store(scalar, k, sem_out2, band)
            scalar.wait_ge(sem_out2, 16 * len(scalar_jobs))

        @block.vector
        def _(vector):
            vector.wait_ge(sem_in, 16)
            for pair in range(n_ch // 2):
                if pair == 1:
                    vector.wait_ge(sem_in2, 16)
                ks = (2 * pair, 2 * pair + 1)
                vs = [cview(k) for k in ks]
                base = POPS * pair

                for l in range(2, L_MAX + 1):
                    lb = base + 6 * (l - 3)  # previous level's block base
                    o, ps = OFF[l], S[l - 1]
                    zo = o + ps + l  # z^l slot (last of level l)
                    po = OFF[l - 1]
                    pzo = po + ps - 1
                    pto = po + ps - l
                    # z^l = z * z^(l-1)
                    for j, k in enumerate(ks):
                        vv, sz = vs[j], sizes[k]
                        if l == 2:
                            zin = dcomp(k, 3, 4)
                        else:
                            vector.wait_ge(sem_v, lb + 1 + j)
                            zin = vv[:, pzo : pzo + 1, :]
                        vector.tensor_mul(
                            out=vv[:, zo : zo + 1, :],
                            in0=zin,
                            in1=dcomp(k, 3, 4),
                        ).then_inc(sem_v, 1)
                    # y * (a=0 tail of L_{l-1}: its last l entries)
                    for j, k in enumerate(ks):
                        vv, sz = vs[j], sizes[k]
                        if l == 2:
                            ysrc = dcomp(k, 2, 4)
                        else:
                            vector.wait_ge(sem_v, lb + 3 + j)
                            ysrc = vv[:, pto : pto + l, :]
                        vector.tensor_mul(
                            out=vv[:, o + ps : o + ps + l, :],
                            in0=ysrc,
                            in1=dcomp(k, 2, 3).broadcast_to([128, l, sz]),
                        ).then_inc(sem_v, 1)
                    # x * (all of L_{l-1})
                    for j, k in enumerate(ks):
                        vv, sz = vs[j], sizes[k]
                        if l == 2:
                            prev = dcomp(k, 1, 4)
                        else:
                            vector.wait_ge(sem_v, lb + 5 + j)
                            prev = vv[:, po : po + ps, :]
                        vector.tensor_mul(
                            out=vv[:, o : o + ps, :],
                            in0=prev,
                            in1=dcomp(k, 1, 2).broadcast_to([128, ps, sz]),
                        ).then_inc(sem_v, 1)

    return nc


def kernel(dr, _trace=False, _trace_cores=None):
    import ml_dtypes
    from concourse.bass_utils import run_bass_kernel_spmd

    dr = np.asarray(dr, dtype=np.float32)
    n = dr.shape[0]
    # Overlapping shards: core i processes rows [i*step, i*step + 25088) so
    # the 704 rows of pad-to-25088 waste is spread evenly (88 rows per core)
    # instead of all landing on the last core.
    step = n // N_CORES
    assert step <= ROWS_PER_CORE and (N_CORES - 1) * step + ROWS_PER_CORE >= n
    total = (N_CORES - 1) * step + ROWS_PER_CORE
    drb = dr.astype(ml_dtypes.bfloat16)
    dr4 = np.zeros((total, 4), dtype=ml_dtypes.bfloat16)
    dr4[:, 0] = 1.0
    dr4[:n, 1:] = drb

    in_maps = [
        {"dr4": np.ascontiguousarray(dr4[i * step : i * step + ROWS_PER_CORE])}
        for i in range(N_CORES)
    ]
    nc = _build_nc()
    res = run_bass_kernel_spmd(
        nc,
        in_maps,
        core_ids=list(range(N_CORES)),
        trace=_trace,
        trace_cores=_trace_cores,
    )
    kernel.last_result = res

    # untangle the monomial-major per-chunk dumps into [25088, 80] per core
    starts = np.concatenate([[0], np.cumsum(SIZES)[:-1]])
    per_core = []
    for i in range(N_CORES):
        arr = np.asarray(res.results[i]["out"])  # [128, G*U] bf16
        blocks = []
        for k, sz in enumerate(SIZES):
            b = arr[:, starts[k] * U : (starts[k] + sz) * U]
            blocks.append(b.reshape(128, U, sz).transpose(0, 2, 1))
        per_core.append(
            np.concatenate(blocks, axis=1).reshape(ROWS_PER_CORE, U)
        )
    dev = np.concatenate(
        [per_core[i][:step] for i in range(N_CORES - 1)]
        + [per_core[N_CORES - 1][: ROWS_PER_CORE - 88]],
        axis=0,
    )
    # unshard: assemble the 84 unique monomials (host-known [1,x,y,z] +
    # 80 device columns), upcast, and expand to the 1093 output columns
    uniq = np.empty((n, 84), dtype=np.float32)
    uniq[:, 0] = 1.0
    uniq[:, 1:4] = drb.astype(np.float32)  # match device bf16 rounding
    uniq[:, 4:] = dev[:n].astype(np.float32)
    return uniq[:, IDX]


# revision 17
# speedup vs baseline: 1.2028x; 1.0373x over previous
"""Angular tensor-product basis expansion on 8 Trainium2 NeuronCores.

Input dr [200000, 3] f32 -> output [200000, 1093] f32 where the columns are
the levels of the recursive tensor-product basis: level l has 3^l entries,
entry (j*3+k) of level l = level_{l-1}[j] * dr[k].

The tensor-product basis is symmetric: the level-l entry with base-3 digits
(d1..dl) equals x^a y^b z^c where a,b,c count the digits equal to 0,1,2.
Level l therefore has only C(l+2,2) distinct values; across levels 0..6 the
1093 columns take just 84 distinct monomial values per row, and 4 of those
(1, x, y, z) are the input itself. The device computes exactly the 80
level-2..6 monomials per row (bf16) and the host expands them to the full
1093 fp32 columns with a precomputed index gather during the unshard step --
cutting HBM store traffic per core from 109.7 MB (fp32 full) to 4.0 MB, a
27x reduction on the memory-bound store stream.

Monomial ordering (so each level needs only 3 strided DVE ops):
  L_1 = [x, y, z];  L_l = [x * L_{l-1} (all)] ++ [y * (last l of L_{l-1})]
                          ++ [z * (last 1 of L_{l-1})]
By induction the a=0 monomials are exactly the trailing l+1 entries of L_l,
so the y-source (a=0 entries of L_{l-1}) is a contiguous tail slice.
Level 2 reads x,y,z straight from the input tile.

Measured DVE cost: op duration ~ n_runs * (run_len * ~1.05ns + ~2.6ns) with
a ~170ns issue floor, where a "run" is the op's innermost contiguous span.
So each chunk's scratch tile is MONOMIAL-MAJOR ([partition, monomial, row]):
every op's inner run is the row dimension (28..70 elems), not the 1..21
monomials a row-major layout would give -- this matters most for the tiny
z-power and y-tail ops, which in row-major cost ~2.6-5ns per element.
Chunks are processed in interleaved pairs so every RAW wait (ops are not
interlocked; each op's completion tick is what dependents wait on) lands
4+ ops after its producer and is pre-satisfied. Store DMAs are contiguous
per-partition dumps of the tile (the host untangles the per-chunk transpose
during the gather), alternating between the sync and scalar DMA queues,
which together sustain >400 GB/s. A second compute engine does not help:
vector and gpsimd contend for the same bandwidth and gpsimd's per-op cost
is ~3x.

Data-parallel row sharding across 8 cores (25000 rows each, padded to
25088 = 128 partitions * 196 rows). Partition p owns the contiguous row
chunk [p*196, (p+1)*196).

Raw Bass (no Tile) so DMA instructions carry at most one semaphore wait --
walrus rejects HWDGE direct DMAs with more than one sync-wait command.
"""

import numpy as np

L_MAX = 6
N_CORES = 8
G = 196  # rows owned by one partition
ROWS_PER_CORE = 128 * G  # 25088
S = [1, 3, 6, 10, 15, 21, 28]  # unique monomials per level
OFF = [0, 0, 0, 6, 16, 31, 52]  # device column offset of level l (l>=2)
U = 80  # stored monomials (levels 2..6)
SIZES = (98, 98)  # rows per chunk; consecutive pairs interleave
POPS = 30  # vector ops per pair: 5 levels * (z_a z_b B_a B_b A_a A_b)


def _index_map():
    """Map each of the 1093 reference columns to unique-monomial index 0..83
    (0..3 = [1, x, y, z] host-side; 4+i = device column i)."""
    mono = [[(0, 0, 0)]]
    for l in range(1, L_MAX + 1):
        prev = mono[-1]
        cur = [(a + 1, b, c) for (a, b, c) in prev]
        cur += [(a, b + 1, c) for (a, b, c) in prev[-l:]]
        a, b, c = prev[-1]
        cur += [(a, b, c + 1)]
        mono.append(cur)
    lookup = {t: i for i, t in enumerate(t for lst in mono for t in lst)}
    idx = []
    for l in range(L_MAX + 1):
        for j in range(3**l):
            a = b = c = 0
            for _ in range(l):
                d = j % 3
                j //= 3
                a += d == 0
                b += d == 1
                c += d == 2
            idx.append(lookup[(a, b, c)])
    return np.asarray(idx, dtype=np.intp)


IDX = _index_map()  # [1093] into [1, x, y, z, device cols 0..79]


def _build_nc(sizes=SIZES):
    import concourse.bass as bass
    import concourse.mybir as mybir

    bf16 = mybir.dt.bfloat16
    g = sum(sizes)
    assert g == G
    rows = 128 * g
    starts = np.concatenate([[0], np.cumsum(sizes)[:-1]])
    n_ch = len(sizes)
    assert n_ch % 2 == 0

    nc = bass.Bass()
    dr4 = nc.declare_dram_parameter("dr4", [rows, 4], bf16, isOutput=False)
    # per chunk k the dump is [p, monomial c, row t]: element (p, k, c, t)
    # lands at out[p, starts[k]*U + c*sizes[k] + t]; host untangles
    out = nc.declare_dram_parameter("out", [128, g * U], bf16, isOutput=True)

    dr4_v = dr4[:, :].rearrange("(p g) c -> p (g c)", p=128)  # [128, g*4]

    from contextlib import ExitStack

    with ExitStack() as stack:
        drt = stack.enter_context(nc.sbuf_tensor("drt", [128, g * 4], bf16))
        uq = stack.enter_context(nc.sbuf_tensor("uq", [128, g * U], bf16))
        sem_in = stack.enter_context(nc.semaphore("sem_in"))
        sem_in2 = stack.enter_context(nc.semaphore("sem_in2"))
        sem_out = stack.enter_context(nc.semaphore("sem_out"))
        sem_out2 = stack.enter_context(nc.semaphore("sem_out2"))
        sem_v = stack.enter_context(nc.semaphore("sem_v"))
        block = stack.enter_context(nc.Block(no_gpsimd_drain=True))

        def cview(k):
            # chunk k scratch as [p, monomial, row]
            st, sz = starts[k], sizes[k]
            return uq[:, st * U : (st + sz) * U].rearrange(
                "p (c t) -> p c t", c=U
            )

        def dcomp(k, c0, c1):
            # input components [c0, c1) for chunk k as [p, comp, row]
            st, sz = starts[k], sizes[k]
            return drt[:, st * 4 : (st + sz) * 4].rearrange(
                "p (t c) -> p c t", c=4
            )[:, c0:c1, :]

        # pair op order per level: z_a z_b B_a B_b A_a A_b (6 per level);
        # chunk a completes at its A6 (pair index 29), chunk b at 30
        def cthr(k):
            return POPS * (k // 2) + 29 + (k % 2)

        def store(q, k, sem, band=None):
            # band=None: whole chunk, waits full completion. Banded stores
            # stream a finished level range early: "lo4" = levels 2..4
            # (ready at the chunk's A4 op), "lo5" = level 5 (at A5),
            # "hi" = level 6 (at full completion).
            st, sz = starts[k], sizes[k]
            base, pj = POPS * (k // 2), k % 2
            lo, hi = st * U, (st + sz) * U
            if band == "lo4":
                hi = lo + OFF[5] * sz
                q.wait_ge(sem_v, base + 17 + pj)
            elif band == "lo5":
                lo, hi = lo + OFF[5] * sz, lo + OFF[6] * sz
                q.wait_ge(sem_v, base + 23 + pj)
            elif band == "a6":
                # level-6 x-block, emitted before the final z/y tail ops
                lo, hi = lo + OFF[6] * sz, lo + (OFF[6] + S[5]) * sz
                q.wait_ge(sem_v, base + 25 + pj)
            elif band == "yz6":
                lo = lo + (OFF[6] + S[5]) * sz
                q.wait_ge(sem_v, cthr(k))
            else:
                if band == "hi":
                    lo = lo + OFF[6] * sz
                q.wait_ge(sem_v, cthr(k))
            q.dma_start(
                out=out[:, lo:hi], in_=uq[:, lo:hi]
            ).then_inc(sem, 16)

        # the final pair's chunks finish last (nothing left to overlap
        # them with), so stream their finished level bands early and only
        # the small level-6 block after completion, one chunk per queue
        sync_jobs = [(k, None) for k in range(0, n_ch - 2, 2)]
        sync_jobs += [
            (n_ch - 2, "lo4"),
            (n_ch - 2, "lo5"),
            (n_ch - 2, "a6"),
            (n_ch - 2, "yz6"),
        ]
        scalar_jobs = [(k, None) for k in range(1, n_ch - 2, 2)]
        scalar_jobs += [
            (n_ch - 1, "lo4"),
            (n_ch - 1, "lo5"),
            (n_ch - 1, "a6"),
            (n_ch - 1, "yz6"),
        ]

        @block.sync
        def _(sync):
            if n_ch > 2:
                c0 = (sizes[0] + sizes[1]) * 4  # first-pair input columns
                sync.dma_start(out=drt[:, :c0], in_=dr4_v[:, :c0]).then_inc(
                    sem_in, 16
                )
                sync.dma_start(out=drt[:, c0:], in_=dr4_v[:, c0:]).then_inc(
                    sem_in2, 16
                )
            else:
                sync.dma_start(out=drt[:, :], in_=dr4_v).then_inc(sem_in, 16)
            for k, band in sync_jobs:
                store(sync, k, sem_out, band)
            sync.wait_ge(sem_out, 16 * len(sync_jobs))

        @block.scalar
        def _(scalar):
            for k, band in scalar_jobs:
                store(scalar, k, sem_out2, band)
            scalar.wait_ge(sem_out2, 16 * len(scalar_jobs))

        @block.vector
        def _(vector):
            vector.wait_ge(sem_in, 16)
            for pair in range(n_ch // 2):
                if pair == 1:
                    vector.wait_ge(sem_in2, 16)
                ks = (2 * pair, 2 * pair + 1)
                vs = [cview(k) for k in ks]
                base = POPS * pair

                for l in range(2, L_MAX + 1):
                    lb = base + 6 * (l - 3)  # previous level's block base
                    o, ps = OFF[l], S[l - 1]
                    zo = o + ps + l  # z^l slot (last of level l)
                    po = OFF[l - 1]
                    pzo = po + ps - 1
                    pto = po + ps - l

                    def z_op(j, k):
                        vv, sz = vs[j], sizes[k]
                        if l == 2:
                            zin = dcomp(k, 3, 4)
                        else:
                            vector.wait_ge(sem_v, lb + 1 + j)
                            zin = vv[:, pzo : pzo + 1, :]
                        vector.tensor_mul(
                            out=vv[:, zo : zo + 1, :],
                            in0=zin,
                            in1=dcomp(k, 3, 4),
                        ).then_inc(sem_v, 1)

                    def y_op(j, k):
                        # y * (a=0 tail of L_{l-1}: its last l entries)
                        vv, sz = vs[j], sizes[k]
                        if l == 2:
                            ysrc = dcomp(k, 2, 4)
                        else:
                            vector.wait_ge(sem_v, lb + 3 + j)
                            ysrc = vv[:, pto : pto + l, :]
                        vector.tensor_mul(
                            out=vv[:, o + ps : o + ps + l, :],
                            in0=ysrc,
                            in1=dcomp(k, 2, 3).broadcast_to([128, l, sz]),
                        ).then_inc(sem_v, 1)

                    def x_op(j, k):
                        # x * (all of L_{l-1})
                        vv, sz = vs[j], sizes[k]
                        if l == 2:
                            prev = dcomp(k, 1, 4)
                        else:
                            vector.wait_ge(sem_v, lb + 5 + j)
                            prev = vv[:, po : po + ps, :]
                        vector.tensor_mul(
                            out=vv[:, o : o + ps, :],
                            in0=prev,
                            in1=dcomp(k, 1, 2).broadcast_to([128, ps, sz]),
                        ).then_inc(sem_v, 1)

                    # last level: big x-ops first so their store band can
                    # start draining while the tiny z/y tail ops finish
                    ops = (
                        (x_op, z_op, y_op) if l == L_MAX else (z_op, y_op, x_op)
                    )
                    for op in ops:
                        for j, k in enumerate(ks):
                            op(j, k)

    return nc


def kernel(dr, _trace=False, _trace_cores=None):
    import ml_dtypes
    from concourse.bass_utils import run_bass_kernel_spmd

    dr = np.asarray(dr, dtype=np.float32)
    n = dr.shape[0]
    # Overlapping shards: core i processes rows [i*step, i*step + 25088) so
    # the 704 rows of pad-to-25088 waste is spread evenly (88 rows per core)
    # instead of all landing on the last core.
    step = n // N_CORES
    assert step <= ROWS_PER_CORE and (N_CORES - 1) * step + ROWS_PER_CORE >= n
    total = (N_CORES - 1) * step + ROWS_PER_CORE
    drb = dr.astype(ml_dtypes.bfloat16)
    dr4 = np.zeros((total, 4), dtype=ml_dtypes.bfloat16)
    dr4[:, 0] = 1.0
    dr4[:n, 1:] = drb

    in_maps = [
        {"dr4": np.ascontiguousarray(dr4[i * step : i * step + ROWS_PER_CORE])}
        for i in range(N_CORES)
    ]
    nc = _build_nc()
    res = run_bass_kernel_spmd(
        nc,
        in_maps,
        core_ids=list(range(N_CORES)),
        trace=_trace,
        trace_cores=_trace_cores,
    )
    kernel.last_result = res

    # untangle the monomial-major per-chunk dumps into [25088, 80] per core
    starts = np.concatenate([[0], np.cumsum(SIZES)[:-1]])
    per_core = []
    for i in range(N_CORES):
        arr = np.asarray(res.results[i]["out"])  # [128, G*U] bf16
        blocks = []
        for k, sz in enumerate(SIZES):
            b = arr[:, starts[k] * U : (starts[k] + sz) * U]
            blocks.append(b.reshape(128, U, sz).transpose(0, 2, 1))
        per_core.append(
            np.concatenate(blocks, axis=1).reshape(ROWS_PER_CORE, U)
        )
    dev = np.concatenate(
        [per_core[i][:step] for i in range(N_CORES - 1)]
        + [per_core[N_CORES - 1][: ROWS_PER_CORE - 88]],
        axis=0,
    )
    # unshard: assemble the 84 unique monomials (host-known [1,x,y,z] +
    # 80 device columns), upcast, and expand to the 1093 output columns
    uniq = np.empty((n, 84), dtype=np.float32)
    uniq[:, 0] = 1.0
    uniq[:, 1:4] = drb.astype(np.float32)  # match device bf16 rounding
    uniq[:, 4:] = dev[:n].astype(np.float32)
    return uniq[:, IDX]
